# revision 12
# baseline (speedup 1.0000x reference)
"""Multi-head self-attention Trainium2 kernel (Bass/Tile), v2.

Problem: x:(8,256,32,32), 8 heads, head_dim=32, N=H*W=1024.
Sharding: data-parallel over batch B=8 -> one batch element per NeuronCore.

Per-core math (b fixed, X = x[b] as (C=256, N=1024)):
  q = Wq@X + bq ; k = Wk@X + bk ; v = Wv@X + bv      (per-pixel linear)
  S[n,m] = sum_d q[d,n]k[d,m] / sqrt(32)  (per head)
  P = softmax_m(S) ; O[d,n] = sum_m P[n,m] v[d,m] ; out = Wo@O + bo + X

Bias algebra (exact, folded on host):
  - bk contributes q^T bk, constant along the softmax axis -> drops.
  - bq contributes (bq^T k_raw)[m]: folded as an extra row of the K-hat
    projection (row u_h = Wk_h^T bq_h / sqrt32), matched by a ones-row in
    Q-hat -> scores leave the PE fully biased+scaled.
  - bv contributes bv -> folded into residual via xpb = x[b] + (Wo@bv + bo).
  - 1/sqrt(32) folded into Wq-hat and u rows.

v2 engine plan (baseline was 134us; PE-bound with f32r + 26us tail):
  - All PE inputs bf16 (scores 0.42ns/col instead of 0.67) and bf16 DMA.
  - exp (ACT) writes fp8-e4m3 E with bias=-3 (cancels in softmax; keeps
    E < 448 = e4m3 max for any plausible score; scores here are ~N(0,1)).
  - AV uses fp8 DoubleRow matmuls: V-hat (fp8, ones-augmented) stationary
    [128,(2,33)], E-hat moving [128,(2,512)] -> two m-chunks per matmul at
    0.5 cycles/row: AV drops 34us -> ~7us of PE time.
  - ACT exp stream (64 ops x ~1.1us = 70us) is the roofline; PE (~45us),
    DVE (~30us) and DMA hide under it.
  - Tail shrunk: per-half normalize (reciprocal_approx_fast), heads 0-3
    normalized + their half of the output projection run mid-kernel
    (emitted inside p=2 so the PE queue never head-of-line blocks).
"""

import math

import numpy as np
import ml_dtypes

import concourse.bass as bass
import concourse.mybir as mybir
import concourse.tile as tile
from concourse import bacc
from concourse.bass_utils import run_bass_kernel_spmd

F32 = mybir.dt.float32
F32R = mybir.dt.float32r
BF16 = mybir.dt.bfloat16
FP8 = mybir.dt.float8e4
EXP = mybir.ActivationFunctionType.Exp
DR = mybir.MatmulPerfMode.DoubleRow

NH = 8          # heads
HD = 32         # head dim
C = 256         # channels
N = 1024        # H*W
NCORES = 8
EBIAS = -3.0    # exp(s + EBIAS): cancels in softmax, keeps e4m3 in range

BF16NP = np.dtype(ml_dtypes.bfloat16)

DEBUG_DUMPS = False

_NC = None          # cached compiled Bass module
LAST_RESULTS = None  # BassKernelResults of most recent run (for test.py)


def _r(ap):
    return ap.bitcast(F32R)


def _emit(tc, io):
    nc = tc.nc
    import contextlib

    ctx = contextlib.ExitStack()
    with ctx:
        pers = ctx.enter_context(tc.tile_pool(name="pers", bufs=1))
        etp = ctx.enter_context(tc.tile_pool(name="etp", bufs=3))
        psp = ctx.enter_context(tc.tile_pool(name="psp", bufs=2, space="PSUM"))

        def ptile(name, shape, dtype=F32):
            return pers.tile(shape, dtype, tag=name, name=name)

        # warm the ACT exp table immediately (PSEUDO_LOAD ~1.3us otherwise
        # serializes with the first real exp); gpsimd memset is the fastest
        # producer for the input tile.
        warm = ptile("warm", [1, 8])
        nc.gpsimd.memset(warm[:], 0.0)
        nc.scalar.activation(warm[:], warm[:], EXP)
        EB = ptile("EB", [128, 1])  # per-partition exp bias (constant)
        nc.gpsimd.memset(EB[:], EBIAS)

        # ---------------- load inputs ----------------
        X = [ptile(f"X{i}", [128, N], BF16) for i in range(2)]
        XPB = [ptile(f"XPB{i}", [128, N]) for i in range(2)]
        WQT = [ptile(f"WQT{i}", [128, 512], BF16) for i in range(2)]
        WKT = [ptile(f"WKT{i}", [128, 512], BF16) for i in range(2)]
        WVT = [ptile(f"WVT{i}", [128, C], BF16) for i in range(2)]
        WOT = [ptile(f"WOT{i}", [128, C], BF16) for i in range(2)]
        OH = ptile("OH", [4, C], BF16)
        for i in range(2):
            sl = slice(i * 128, (i + 1) * 128)
            nc.sync.dma_start(X[i][:], io["xb"][sl, :])
            nc.sync.dma_start(WQT[i][:], io["wqt"][sl, :])
            nc.sync.dma_start(WKT[i][:], io["wkt"][sl, :])
            nc.sync.dma_start(WVT[i][:], io["wvt"][sl, :])
        nc.sync.dma_start(OH[:], io["oh"][:, :])
        for i in range(2):
            sl = slice(i * 128, (i + 1) * 128)
            nc.sync.dma_start(WOT[i][:], io["wot"][sl, :])
            nc.sync.dma_start(XPB[i][:], io["xpb"][sl, :])

        # ---------------- Q-hat / K-hat projections ----------------
        # padded channel space: head h -> rows 64h..64h+33 (4 tiles of 128)
        Qh = [ptile(f"Qh{t}", [128, N], BF16) for t in range(4)]
        Kh = [ptile(f"Kh{t}", [128, N], BF16) for t in range(4)]
        for t in range(4):
            for dst, w in ((Qh, WQT), (Kh, WKT)):
                pp = psp.tile([128, N], F32, tag="big", bufs=2, name=f"pp_{t}")
                for jn in range(2):
                    for kc in range(2):
                        nc.tensor.matmul(
                            pp[:, jn * 512 : (jn + 1) * 512],
                            w[kc][:, t * 128 : (t + 1) * 128],
                            X[kc][:, jn * 512 : (jn + 1) * 512],
                            start=(kc == 0),
                            stop=(kc == 1),
                        )
                nc.vector.tensor_copy(dst[t][:], pp[:])
            # ones rows for Q-hat (row 32 of each 64-row slab)
            nc.gpsimd.memset(Qh[t][32:33, :], 1.0)
            nc.gpsimd.memset(Qh[t][96:97, :], 1.0)

        # ---------------- V-hat (fp8, ones-augmented, chunk-paired) --------
        # VH[c2]: [128, (i=2, h=8, c=48)]; [:,i,h,0:32] = V^T rows of m-chunk
        # 2*c2+i, head h; [:,i,h,32] = 1.0 (AV then emits the softmax
        # denominator for free); cols 33-47 pad the head slot to 48 so every
        # DoubleRow ldweights AP start/stride is 16B-aligned (ISA rule).
        # fp8 e4m3: |v| ~ N(0,1), rel err ~3%.
        VH = [ptile(f"VH{c2}", [128, 2 * NH * 48], FP8) for c2 in range(4)]
        for c2 in range(4):
            vh4 = VH[c2].rearrange("p (i h c) -> p i h c", i=2, c=48)
            nc.gpsimd.memset(vh4[:, :, :, 32:33], 1.0)
        for mc in range(8):
            pv = psp.tile([128, C], F32, tag="big", bufs=2, name=f"pv_{mc}")
            for kc in range(2):
                nc.tensor.matmul(
                    pv[:],
                    X[kc][:, mc * 128 : (mc + 1) * 128],
                    WVT[kc][:],
                    start=(kc == 0),
                    stop=(kc == 1),
                )
            vh4 = VH[mc // 2].rearrange("p (i h c) -> p i h c", i=2, c=48)
            nc.vector.tensor_copy(
                vh4[:, mc % 2, :, 0:32], pv.rearrange("p (h d) -> p h d", d=32)
            )

        # ---------------- attention ----------------
        O1u = [ptile(f"O1u{t}", [128, N]) for t in range(2)]
        O1 = [ptile(f"O1{t}", [128, N], BF16) for t in range(2)]
        # per-half denominator tiles, all at base partition 0:
        # reciprocal_approx_fast (custom DVE op) misbehaves on HW when the
        # AP starts at a nonzero partition.
        ESUM = [ptile(f"ESUM{t}", [4, N]) for t in range(2)]
        RECIP = [ptile(f"RECIP{t}", [4, N]) for t in range(2)]
        RECIPB = [ptile(f"RECIPB{t}", [4, N], BF16) for t in range(2)]
        OUTF = [ptile(f"OUTF{t}", [128, N]) for t in range(2)]

        def half_done(t):
            """Normalize heads 4t..4t+3 and do their half of the output
            projection (SBUF-accumulated so no PSUM is pinned)."""
            with nc.allow_low_precision("approx recip of O(100) softmax sums"):
                nc.vector.reciprocal_approx_fast(RECIP[t][:, :], ESUM[t][:, :])
            nc.vector.tensor_copy(RECIPB[t][:, :], RECIP[t][:, :])
            pr = psp.tile([128, N], F32, tag="big", bufs=2, name=f"pr_{t}")
            for jn in range(2):
                js = slice(jn * 512, (jn + 1) * 512)
                nc.tensor.matmul(
                    pr[:, js],
                    OH[0:4, t * 128 : (t + 1) * 128],
                    RECIPB[t][0:4, js],
                    start=True,
                    stop=True,
                )
            nc.vector.tensor_mul(O1[t][:], O1u[t][:], pr[:])
            # output-projection contribution of channel block t
            for mo in range(2):
                for jn in range(2):
                    js = slice(jn * 512, (jn + 1) * 512)
                    po = psp.tile(
                        [128, 512], F32, tag="big", bufs=2, name=f"po_{t}_{mo}_{jn}"
                    )
                    nc.tensor.matmul(
                        po[:],
                        WOT[t][:, mo * 128 : (mo + 1) * 128],
                        O1[t][:, js],
                        start=True,
                        stop=True,
                    )
                    if t == 0:
                        nc.vector.tensor_add(OUTF[mo][:, js], po[:], XPB[mo][:, js])
                    else:
                        nc.vector.tensor_add(OUTF[mo][:, js], po[:], OUTF[mo][:, js])
            if t == 1:
                for mo in range(2):
                    nc.sync.dma_start(
                        io["out"][mo * 128 : (mo + 1) * 128, :], OUTF[mo][:]
                    )

        for p in range(4):  # head pairs (2p, 2p+1) in Qh/Kh tile p
            # DoubleRow dst must start at partition 0 -> one PSUM tile per
            # (jn, hh) instead of column-packing two heads into one bank.
            psO = [
                [
                    psp.tile([33, 512], F32, tag="psO", bufs=4, name=f"psO_{p}_{jn}_{hh}")
                    for hh in range(2)
                ]
                for jn in range(2)
            ]
            for c2 in range(4):  # m-chunk pairs (2c2, 2c2+1)
                # ET: [128, (i=2, hh=2, n=1024)] fp8 exp of scores
                et = etp.tile([128, 2 * 2 * N], FP8, tag="et", name=f"et_{p}_{c2}")
                et4 = et.rearrange("p (i hh n) -> p i hh n", i=2, hh=2)
                for i in range(2):
                    mc = 2 * c2 + i
                    for jn in range(2):
                        ps = psp.tile(
                            [128, N], F32, tag="big", bufs=2, name=f"ps_{p}_{mc}_{jn}"
                        )
                        for hh in range(2):  # array rows 0-32 / 64-96
                            base = 64 * hh
                            nc.tensor.matmul(
                                ps[:, hh * 512 : (hh + 1) * 512],
                                Kh[p][base : base + 33, mc * 128 : (mc + 1) * 128],
                                Qh[p][base : base + 33, jn * 512 : (jn + 1) * 512],
                                start=True,
                                stop=True,
                            )
                        nc.scalar.activation(
                            et4[:, i, :, jn * 512 : (jn + 1) * 512],
                            ps.rearrange("p (hh n) -> p hh n", hh=2),
                            EXP,
                            bias=EB[:],
                        )
                vh3 = VH[c2].rearrange("p (i c) -> p i c", c=NH * 48)
                for jn in range(2):  # fp8 DoubleRow AV: 2 m-chunks per matmul
                    for hh in range(2):
                        h = 2 * p + hh
                        nc.tensor.matmul(
                            psO[jn][hh][:, :],
                            vh3[:, :, 48 * h : 48 * h + 33],
                            et4[:, :, hh, jn * 512 : (jn + 1) * 512],
                            start=(c2 == 0),
                            stop=(c2 == 3),
                            perf_mode=DR,
                        )
                if p == 2 and c2 == 1:
                    # heads 0-3 finished back in p=1; by now their ESUM DMAs
                    # have landed, so this block never stalls the PE queue.
                    half_done(0)
            for jn in range(2):
                js = slice(jn * 512, (jn + 1) * 512)
                for hh in range(2):
                    h = 2 * p + hh
                    ost = etp.tile(
                        [33, 512], F32, tag="ost", bufs=4, name=f"ost_{p}_{jn}_{hh}"
                    )
                    nc.vector.tensor_copy(ost[:], psO[jn][hh][:, :])
                    t, r = h // 4, 32 * (h % 4)
                    nc.sync.dma_start(O1u[t][r : r + 32, js], ost[0:32, :])
                    er = h % 4
                    nc.sync.dma_start(ESUM[t][er : er + 1, js], ost[32:33, :])

        half_done(1)

        if DEBUG_DUMPS:
            for nm, t in [
                ("dQh0", Qh[0]), ("dKh0", Kh[0]),
                ("dO1u0", O1u[0]), ("dO1u1", O1u[1]),
                ("dO10", O1[0]), ("dOUTF0", OUTF[0]),
            ]:
                nc.sync.dma_start(io[nm][:, :], t[:])
            for t2 in range(2):
                nc.sync.dma_start(io["dESUM"][4 * t2 : 4 * t2 + 4, :], ESUM[t2][:, :])
                nc.sync.dma_start(io["dRECIP"][4 * t2 : 4 * t2 + 4, :], RECIP[t2][:, :])
            vh4d = VH[0].rearrange("p (i h c) -> p i h c", i=2, c=48)
            nc.sync.dma_start(
                io["dVH0"].rearrange("p (i h c) -> p i h c", i=2, c=33),
                vh4d[:, :, :, 0:33],
            )


def build_nc():
    nc = bacc.Bacc("TRN2", target_bir_lowering=False, debug=False)
    io = {}
    for name, shape, dt_ in [
        ("xb", (C, N), BF16),
        ("xpb", (C, N), F32),
        ("wqt", (C, 512), BF16),
        ("wkt", (C, 512), BF16),
        ("wvt", (C, C), BF16),
        ("wot", (C, C), BF16),
        ("oh", (4, C), BF16),
    ]:
        io[name] = nc.dram_tensor(name, shape, dt_, kind="ExternalInput").ap()
    io["out"] = nc.dram_tensor("out", (C, N), F32, kind="ExternalOutput").ap()
    if DEBUG_DUMPS:
        for nm, shape, dt_ in [
            ("dQh0", (128, N), BF16), ("dKh0", (128, N), BF16),
            ("dVH0", (128, 2 * NH * 33), FP8), ("dESUM", (8, N), F32),
            ("dO1u0", (128, N), F32), ("dO1u1", (128, N), F32),
            ("dRECIP", (8, N), F32), ("dO10", (128, N), BF16),
            ("dOUTF0", (128, N), F32),
        ]:
            io[nm] = nc.dram_tensor(nm, shape, dt_, kind="ExternalOutput").ap()
    with tile.TileContext(nc) as tc:
        _emit(tc, io)
    nc.finalize()  # Bacc passes: wait-splitting (1-wait limit), reg alloc
    return nc


def host_prep(x, Wq, bq, Wk, bk, Wv, bv, Wo, bo):
    """Build per-core input maps (numpy only)."""
    x = np.ascontiguousarray(np.asarray(x, np.float32))
    Wq, bq = np.asarray(Wq, np.float32), np.asarray(bq, np.float32)
    Wk = np.asarray(Wk, np.float32)
    Wv, bv = np.asarray(Wv, np.float32), np.asarray(bv, np.float32)
    Wo, bo = np.asarray(Wo, np.float32), np.asarray(bo, np.float32)
    s = 1.0 / math.sqrt(HD)

    wqt = np.zeros((C, 512), np.float32)
    wkt = np.zeros((C, 512), np.float32)
    for h in range(NH):
        hs = slice(HD * h, HD * (h + 1))
        wqt[:, 64 * h : 64 * h + 32] = Wq[hs, :].T * s
        wkt[:, 64 * h : 64 * h + 32] = Wk[hs, :].T
        wkt[:, 64 * h + 32] = (Wk[hs, :].T @ bq[hs]) * s
    wvt = np.ascontiguousarray(Wv.T)
    wot = np.ascontiguousarray(Wo.T)
    bo2 = Wo @ bv + bo
    # oh[j//32, 128t + j] = 1: broadcasts RECIP row (head index within the
    # half) onto that head's 32 output partitions; same pattern per half.
    oh = np.zeros((4, C), np.float32)
    for t in range(2):
        for j in range(128):
            oh[j // 32, t * 128 + j] = 1.0

    wqt = wqt.astype(BF16NP)
    wkt = wkt.astype(BF16NP)
    wvt = wvt.astype(BF16NP)
    wot = wot.astype(BF16NP)

    B = x.shape[0]
    in_maps = []
    for b in range(B):
        xb = np.ascontiguousarray(x[b].reshape(C, N))
        in_maps.append(
            {
                "xb": xb.astype(BF16NP),
                "xpb": np.ascontiguousarray(xb + bo2[:, None]),
                "wqt": wqt,
                "wkt": wkt,
                "wvt": wvt,
                "wot": wot,
                "oh": oh.astype(BF16NP),
            }
        )
    return in_maps


def kernel(x, Wq, bq, Wk, bk, Wv, bv, Wo, bo):
    global _NC, LAST_RESULTS
    if _NC is None:
        _NC = build_nc()
    in_maps = host_prep(x, Wq, bq, Wk, bk, Wv, bv, Wo, bo)
    res = run_bass_kernel_spmd(_NC, in_maps, core_ids=list(range(NCORES)))
    LAST_RESULTS = res
    out = np.stack([r["out"] for r in res.results], axis=0)
    return out.reshape(NCORES, C, 32, 32).astype(np.float32)


if __name__ == "__main__":
    # smoke: random inputs through the kernel
    rng = np.random.default_rng(0)
    ins = {
        "x": rng.standard_normal((8, C, 32, 32), dtype=np.float32),
        "Wq": rng.standard_normal((C, C), dtype=np.float32) / 16,
        "bq": rng.standard_normal(C).astype(np.float32) * 0.01,
        "Wk": rng.standard_normal((C, C), dtype=np.float32) / 16,
        "bk": rng.standard_normal(C).astype(np.float32) * 0.01,
        "Wv": rng.standard_normal((C, C), dtype=np.float32) / 16,
        "bv": rng.standard_normal(C).astype(np.float32) * 0.01,
        "Wo": rng.standard_normal((C, C), dtype=np.float32) / 16,
        "bo": rng.standard_normal(C).astype(np.float32) * 0.01,
    }
    out = kernel(**ins)
    print("out", out.shape, out.dtype, float(np.abs(out).mean()))


# revision 14
# speedup vs baseline: 1.1505x; 1.1505x over previous
"""Multi-head self-attention Trainium2 kernel (Bass/Tile), v4.

Problem: x:(8,256,32,32), 8 heads, head_dim=32, N=H*W=1024.
Sharding: data-parallel over batch B=8 -> one batch element per NeuronCore.

Per-core math (b fixed, X = x[b] as (C=256, N=1024)):
  q = Wq@X + bq ; k = Wk@X + bk ; v = Wv@X + bv      (per-pixel linear)
  S[n,m] = sum_d q[d,n]k[d,m] / sqrt(32)  (per head)
  P = softmax_m(S) ; O[d,n] = sum_m P[n,m] v[d,m] ; out = Wo@O + bo + X

Bias algebra (exact, folded on host):
  - bk contributes q^T bk, constant along the softmax axis -> drops.
  - bq contributes (bq^T k_raw)[m]: folded as an extra row of the K-hat
    projection (row u_h = Wk_h^T bq_h / sqrt32), matched by a ones-row in
    Q-hat -> scores leave the PE fully biased+scaled.
  - bv contributes bv -> folded into residual via xpb = x[b] + (Wo@bv + bo).
  - 1/sqrt(32) folded into Wq-hat and u rows.

v4 schedule (HW findings: exp on ACT is a ~69us floor; the PE drops to the
mid p-state after ANY idle, doubling matmul time; row-disjoint paired score
matmuls overlap on the PE; fp8 DoubleRow gives no real HW speedup):
  - everything bf16 into the PE; exp ACT ops [128,1024] write bf16 E.
  - flat software pipeline over the 64 (head-pair, m-chunk, n-half) steps:
    per step emit scores(s) [2 overlapped matmuls], exp(s), AV(s-2)
    [2 matmuls, heads column-packed into one PSUM bank]. Projections,
    V-proj, and the normalize/output-projection of finished halves are
    sprinkled in as PE filler so the PE never idles (keeps max p-state)
    and the ACT exp stream never starves.
  - denominators: ones-column in V-hat -> AV emits them for free; per-half
    reciprocal_approx_fast at partition 0 (the custom DVE op is broken at
    nonzero base partitions on HW); recip broadcast via a tiny matmul.
  - tail is jn-split so the last AV / drain / normalize / out-proj / DMA
    chains of the two n-halves overlap.
"""

import math

import numpy as np
import ml_dtypes

import concourse.bass as bass
import concourse.mybir as mybir
import concourse.tile as tile
from concourse import bacc
from concourse.bass_utils import run_bass_kernel_spmd

F32 = mybir.dt.float32
BF16 = mybir.dt.bfloat16
EXP = mybir.ActivationFunctionType.Exp

NH = 8          # heads
HD = 32         # head dim
C = 256         # channels
N = 1024        # H*W
NCORES = 8

BF16NP = np.dtype(ml_dtypes.bfloat16)

DEBUG_DUMPS = False

_NC = None          # cached compiled Bass module
LAST_RESULTS = None  # BassKernelResults of most recent run (for test.py)


def _emit(tc, io):
    nc = tc.nc
    import contextlib

    ctx = contextlib.ExitStack()
    with ctx:
        pers = ctx.enter_context(tc.tile_pool(name="pers", bufs=1))
        etp = ctx.enter_context(tc.tile_pool(name="etp", bufs=4))
        psp = ctx.enter_context(tc.tile_pool(name="psp", bufs=2, space="PSUM"))

        def ptile(name, shape, dtype=F32):
            return pers.tile(shape, dtype, tag=name, name=name)

        # warm the ACT exp table immediately (table PSEUDO_LOAD ~1.3us would
        # otherwise serialize with the first real exp)
        warm = ptile("warm", [1, 8])
        nc.gpsimd.memset(warm[:], 0.0)
        nc.scalar.activation(warm[:], warm[:], EXP)

        # ---------------- load inputs ----------------
        X = [ptile(f"X{i}", [128, N], BF16) for i in range(2)]
        XPB = [ptile(f"XPB{i}", [128, N]) for i in range(2)]
        WQT = [ptile(f"WQT{i}", [128, 512], BF16) for i in range(2)]
        WKT = [ptile(f"WKT{i}", [128, 512], BF16) for i in range(2)]
        WVT = [ptile(f"WVT{i}", [128, C], BF16) for i in range(2)]
        WOT = [ptile(f"WOT{i}", [128, C], BF16) for i in range(2)]
        OH = ptile("OH", [4, C], BF16)
        for i in range(2):
            sl = slice(i * 128, (i + 1) * 128)
            nc.sync.dma_start(X[i][:], io["xb"][sl, :])
            nc.sync.dma_start(WQT[i][:], io["wqt"][sl, :])
            nc.sync.dma_start(WKT[i][:], io["wkt"][sl, :])
        for i in range(2):
            sl = slice(i * 128, (i + 1) * 128)
            nc.sync.dma_start(WVT[i][:], io["wvt"][sl, :])
            nc.sync.dma_start(WOT[i][:], io["wot"][sl, :])
            nc.sync.dma_start(XPB[i][:], io["xpb"][sl, :])
        nc.sync.dma_start(OH[:], io["oh"][:, :])

        # ---------------- persistent tiles ----------------
        Qh = [ptile(f"Qh{t}", [128, N], BF16) for t in range(4)]
        Kh = [ptile(f"Kh{t}", [128, N], BF16) for t in range(4)]
        VH = [ptile(f"VH{mc}", [128, NH * 33], BF16) for mc in range(8)]
        O1u = [ptile(f"O1u{t}", [128, N]) for t in range(2)]
        O1 = [ptile(f"O1{t}", [128, N], BF16) for t in range(2)]
        # per-half denominator tiles at base partition 0 (HW quirk: the
        # custom reciprocal_approx_fast op needs base partition 0)
        ESUM = [ptile(f"ESUM{t}", [4, N]) for t in range(2)]
        RECIP = [ptile(f"RECIP{t}", [4, N]) for t in range(2)]
        RECIPB = [ptile(f"RECIPB{t}", [4, N], BF16) for t in range(2)]
        OUTF = [ptile(f"OUTF{t}", [128, N]) for t in range(2)]
        for mc in range(8):
            vh3 = VH[mc].rearrange("p (h c) -> p h c", c=33)
            nc.gpsimd.memset(vh3[:, :, 32:33], 1.0)

        # ---------------- emission helpers ----------------
        def qk_proj(t):
            for dst, w in ((Qh, WQT), (Kh, WKT)):
                pp = psp.tile([128, N], F32, tag="big", bufs=3, name=f"pp_{t}")
                for jn in range(2):
                    for kc in range(2):
                        nc.tensor.matmul(
                            pp[:, jn * 512 : (jn + 1) * 512],
                            w[kc][:, t * 128 : (t + 1) * 128],
                            X[kc][:, jn * 512 : (jn + 1) * 512],
                            start=(kc == 0),
                            stop=(kc == 1),
                        )
                nc.vector.tensor_copy(dst[t][:], pp[:])
            nc.gpsimd.memset(Qh[t][32:33, :], 1.0)
            nc.gpsimd.memset(Qh[t][96:97, :], 1.0)

        def v_proj(mc):
            pv = psp.tile([128, C], F32, tag="big", bufs=3, name=f"pv_{mc}")
            for kc in range(2):
                nc.tensor.matmul(
                    pv[:],
                    X[kc][:, mc * 128 : (mc + 1) * 128],
                    WVT[kc][:],
                    start=(kc == 0),
                    stop=(kc == 1),
                )
            vh3 = VH[mc].rearrange("p (h c) -> p h c", c=33)
            nc.vector.tensor_copy(
                vh3[:, :, 0:32], pv.rearrange("p (h d) -> p h d", d=32)
            )

        psO = [None, None]  # current accumulators, per jn

        def scores(p, mc, jn):
            ps = psp.tile([128, N], F32, tag="big", bufs=3, name=f"ps_{p}_{mc}_{jn}")
            for hh in range(2):  # array rows 0-32 / 64-96 run concurrently
                base = 64 * hh
                nc.tensor.matmul(
                    ps[:, hh * 512 : (hh + 1) * 512],
                    Kh[p][base : base + 33, mc * 128 : (mc + 1) * 128],
                    Qh[p][base : base + 33, jn * 512 : (jn + 1) * 512],
                    start=True,
                    stop=True,
                )
            et = etp.tile([128, N], BF16, tag="et", name=f"et_{p}_{mc}_{jn}")
            nc.scalar.activation(et[:], ps[:], EXP)
            return et

        def av(p, mc, jn, et):
            if psO[jn] is None:
                psO[jn] = psp.tile(
                    [97, 512], F32, tag="psO", bufs=2, name=f"psO_{p}_{jn}"
                )
            for hh in range(2):
                h = 2 * p + hh
                nc.tensor.matmul(
                    psO[jn][64 * hh : 64 * hh + 33, :],
                    VH[mc][:, 33 * h : 33 * h + 33],
                    et[:, hh * 512 : (hh + 1) * 512],
                    start=(mc == 0),
                    stop=(mc == 7),
                    tile_position=(0, 64 * hh),
                    skip_group_check=True,
                )

        def drain(p, jn):
            js = slice(jn * 512, (jn + 1) * 512)
            ost = etp.tile([97, 512], F32, tag="ost", bufs=4, name=f"ost_{p}_{jn}")
            nc.vector.tensor_copy(ost[0:33, :], psO[jn][0:33, :])
            nc.vector.tensor_copy(ost[64:97, :], psO[jn][64:97, :])
            for hh in range(2):
                h = 2 * p + hh
                t, r = h // 4, 32 * (h % 4)
                nc.sync.dma_start(
                    O1u[t][r : r + 32, js], ost[64 * hh : 64 * hh + 32, :]
                )
                nc.sync.dma_start(
                    ESUM[t][h % 4 : h % 4 + 1, js], ost[64 * hh + 32 : 64 * hh + 33, :]
                )
            psO[jn] = None

        def recip_half(t, jn):
            js = slice(jn * 512, (jn + 1) * 512)
            with nc.allow_low_precision("approx recip of O(100) softmax sums"):
                nc.vector.reciprocal_approx_fast(RECIP[t][:, js], ESUM[t][:, js])
            nc.vector.tensor_copy(RECIPB[t][:, js], RECIP[t][:, js])

        def norm_half(t, jn):
            js = slice(jn * 512, (jn + 1) * 512)
            pr = psp.tile([128, 512], F32, tag="big", bufs=3, name=f"pr_{t}_{jn}")
            nc.tensor.matmul(
                pr[:],
                OH[0:4, t * 128 : (t + 1) * 128],
                RECIPB[t][0:4, js],
                start=True,
                stop=True,
            )
            nc.vector.tensor_mul(O1[t][:, js], O1u[t][:, js], pr[:])

        def oproj(t, mo, jn):
            js = slice(jn * 512, (jn + 1) * 512)
            po = psp.tile([128, 512], F32, tag="big", bufs=3, name=f"po_{t}_{mo}_{jn}")
            nc.tensor.matmul(
                po[:],
                WOT[t][:, mo * 128 : (mo + 1) * 128],
                O1[t][:, js],
                start=True,
                stop=True,
            )
            if t == 0:
                nc.vector.tensor_add(OUTF[mo][:, js], po[:], XPB[mo][:, js])
            else:
                nc.vector.tensor_add(OUTF[mo][:, js], po[:], OUTF[mo][:, js])

        # ---------------- software-pipelined main loop ----------------
        # filler units keep the PE from idling (p-state); qk2/qk3 are
        # reserved for the head-pair boundaries where the AV lag collapses.
        filler = {
            0: lambda: v_proj(2), 2: lambda: v_proj(3), 4: lambda: v_proj(4),
            6: lambda: v_proj(5), 8: lambda: v_proj(6), 10: lambda: v_proj(7),
            12: lambda: qk_proj(1), 15: lambda: qk_proj(2), 31: lambda: qk_proj(3),
        }

        qk_proj(0)
        v_proj(0)
        v_proj(1)

        steps = [(p, mc, jn) for p in range(4) for mc in range(8) for jn in range(2)]
        pend = []  # (p, mc, jn, et) AV work, emitted with lag 2
        for s, (p, mc, jn) in enumerate(steps):
            if mc == 0 and jn == 0 and p > 0:
                while pend:  # collapse AV lag at the head-pair boundary
                    av(*pend.pop(0))
                drain(p - 1, 0)
                drain(p - 1, 1)
            et = scores(p, mc, jn)
            pend.append((p, mc, jn, et))
            if len(pend) > 2:
                av(*pend.pop(0))
            if s in filler:
                filler.pop(s)()
            # normalize + output-projection of heads 0-3 mid-stream (their
            # drains complete at the p=2 boundary, s=32)
            if s == 34:
                recip_half(0, 0)
                recip_half(0, 1)
            elif s == 37:
                norm_half(0, 0)
            elif s == 38:
                norm_half(0, 1)
            elif s == 40:
                oproj(0, 0, 0)
            elif s == 41:
                oproj(0, 0, 1)
            elif s == 42:
                oproj(0, 1, 0)
            elif s == 43:
                oproj(0, 1, 1)

        # ---------------- tail (jn-split, staggered) ----------------
        while len(pend) > 1:
            av(*pend.pop(0))
        drain(3, 0)
        recip_half(1, 0)
        av(*pend.pop(0))
        drain(3, 1)
        norm_half(1, 0)
        recip_half(1, 1)
        oproj(1, 0, 0)
        norm_half(1, 1)
        oproj(1, 1, 0)
        oproj(1, 0, 1)
        nc.sync.dma_start(io["out"][0:128, :], OUTF[0][:])
        oproj(1, 1, 1)
        nc.sync.dma_start(io["out"][128:256, :], OUTF[1][:])

        if DEBUG_DUMPS:
            for nm, t in [
                ("dQh0", Qh[0]), ("dKh0", Kh[0]),
                ("dO1u0", O1u[0]), ("dO1u1", O1u[1]),
                ("dO10", O1[0]), ("dOUTF0", OUTF[0]),
            ]:
                nc.sync.dma_start(io[nm][:, :], t[:])
            for t2 in range(2):
                nc.sync.dma_start(io["dESUM"][4 * t2 : 4 * t2 + 4, :], ESUM[t2][:, :])
                nc.sync.dma_start(io["dRECIP"][4 * t2 : 4 * t2 + 4, :], RECIP[t2][:, :])


def build_nc():
    nc = bacc.Bacc("TRN2", target_bir_lowering=False, debug=False)
    io = {}
    for name, shape, dt_ in [
        ("xb", (C, N), BF16),
        ("xpb", (C, N), F32),
        ("wqt", (C, 512), BF16),
        ("wkt", (C, 512), BF16),
        ("wvt", (C, C), BF16),
        ("wot", (C, C), BF16),
        ("oh", (4, C), BF16),
    ]:
        io[name] = nc.dram_tensor(name, shape, dt_, kind="ExternalInput").ap()
    io["out"] = nc.dram_tensor("out", (C, N), F32, kind="ExternalOutput").ap()
    if DEBUG_DUMPS:
        for nm, shape, dt_ in [
            ("dQh0", (128, N), BF16), ("dKh0", (128, N), BF16),
            ("dESUM", (8, N), F32),
            ("dO1u0", (128, N), F32), ("dO1u1", (128, N), F32),
            ("dRECIP", (8, N), F32), ("dO10", (128, N), BF16),
            ("dOUTF0", (128, N), F32),
        ]:
            io[nm] = nc.dram_tensor(nm, shape, dt_, kind="ExternalOutput").ap()
    with tile.TileContext(nc) as tc:
        _emit(tc, io)
    nc.finalize()  # Bacc passes: wait-splitting (1-wait limit), reg alloc
    return nc


def host_prep(x, Wq, bq, Wk, bk, Wv, bv, Wo, bo):
    """Build per-core input maps (numpy only)."""
    x = np.ascontiguousarray(np.asarray(x, np.float32))
    Wq, bq = np.asarray(Wq, np.float32), np.asarray(bq, np.float32)
    Wk = np.asarray(Wk, np.float32)
    Wv, bv = np.asarray(Wv, np.float32), np.asarray(bv, np.float32)
    Wo, bo = np.asarray(Wo, np.float32), np.asarray(bo, np.float32)
    s = 1.0 / math.sqrt(HD)

    wqt = np.zeros((C, 512), np.float32)
    wkt = np.zeros((C, 512), np.float32)
    for h in range(NH):
        hs = slice(HD * h, HD * (h + 1))
        wqt[:, 64 * h : 64 * h + 32] = Wq[hs, :].T * s
        wkt[:, 64 * h : 64 * h + 32] = Wk[hs, :].T
        wkt[:, 64 * h + 32] = (Wk[hs, :].T @ bq[hs]) * s
    wvt = np.ascontiguousarray(Wv.T)
    wot = np.ascontiguousarray(Wo.T)
    bo2 = Wo @ bv + bo
    # oh[j//32, 128t + j] = 1: broadcasts RECIP row (head index within the
    # half) onto that head's 32 output partitions; same pattern per half.
    oh = np.zeros((4, C), np.float32)
    for t in range(2):
        for j in range(128):
            oh[j // 32, t * 128 + j] = 1.0

    wqt = wqt.astype(BF16NP)
    wkt = wkt.astype(BF16NP)
    wvt = wvt.astype(BF16NP)
    wot = wot.astype(BF16NP)

    B = x.shape[0]
    in_maps = []
    for b in range(B):
        xb = np.ascontiguousarray(x[b].reshape(C, N))
        in_maps.append(
            {
                "xb": xb.astype(BF16NP),
                "xpb": np.ascontiguousarray(xb + bo2[:, None]),
                "wqt": wqt,
                "wkt": wkt,
                "wvt": wvt,
                "wot": wot,
                "oh": oh.astype(BF16NP),
            }
        )
    return in_maps


def kernel(x, Wq, bq, Wk, bk, Wv, bv, Wo, bo):
    global _NC, LAST_RESULTS
    if _NC is None:
        _NC = build_nc()
    in_maps = host_prep(x, Wq, bq, Wk, bk, Wv, bv, Wo, bo)
    res = run_bass_kernel_spmd(_NC, in_maps, core_ids=list(range(NCORES)))
    LAST_RESULTS = res
    out = np.stack([r["out"] for r in res.results], axis=0)
    return out.reshape(NCORES, C, 32, 32).astype(np.float32)


if __name__ == "__main__":
    # smoke: random inputs through the kernel
    rng = np.random.default_rng(0)
    ins = {
        "x": rng.standard_normal((8, C, 32, 32), dtype=np.float32),
        "Wq": rng.standard_normal((C, C), dtype=np.float32) / 16,
        "bq": rng.standard_normal(C).astype(np.float32) * 0.01,
        "Wk": rng.standard_normal((C, C), dtype=np.float32) / 16,
        "bk": rng.standard_normal(C).astype(np.float32) * 0.01,
        "Wv": rng.standard_normal((C, C), dtype=np.float32) / 16,
        "bv": rng.standard_normal(C).astype(np.float32) * 0.01,
        "Wo": rng.standard_normal((C, C), dtype=np.float32) / 16,
        "bo": rng.standard_normal(C).astype(np.float32) * 0.01,
    }
    out = kernel(**ins)
    print("out", out.shape, out.dtype, float(np.abs(out).mean()))
